# revision 1
# baseline (speedup 1.0000x reference)
"""Trainium2 Bass kernel for a rate-1/2, constraint-length-3 feedforward
convolutional encoder (generator polynomials "101" and "111", MSB-first).

The trellis scan in the reference collapses to elementwise XORs of shifted
input bits (zero initial state):

    out0[t] = u[t] ^ u[t-2]            (poly "101")
    out1[t] = u[t] ^ u[t-1] ^ u[t-2]   (poly "111")

with the codeword interleaved time-major: y[:, 2t] = out0[t], y[:, 2t+1] = out1[t].

XOR on {0,1} floats is computed arithmetically: x ^ y = (x - y)^2.

Sharding: pure data parallel over the batch dim across 8 NeuronCores.
The kernel is DMA-bound (3 MiB of HBM traffic per 1 MiB of input); the
compute (2 vector + 2 scalar ops per tile) hides entirely under the DMA.
"""

import numpy as np

N_CORES = 8
B, K = 8192, 2048
N_OUT = 2
SHARD_B = B // N_CORES  # 1024 codewords per core
P = 128                 # SBUF partitions

_compiled = {}


def _build_nc():
    import concourse.bass as bass  # noqa: F401
    import concourse.tile as tile
    from concourse import bacc, mybir

    nc = bacc.Bacc(
        "TRN2",
        target_bir_lowering=False,
        debug=False,
        enable_asserts=False,
    )
    x = nc.dram_tensor("x", [SHARD_B, K], mybir.dt.float32, kind="ExternalInput").ap()
    y = nc.dram_tensor(
        "y", [SHARD_B, N_OUT * K], mybir.dt.float32, kind="ExternalOutput"
    ).ap()

    n_groups = SHARD_B // P  # 8 row-groups of 128
    N_SLOTS = 6

    with tile.TileContext(nc) as tc:
        with (
            tc.tile_pool(name="xin", bufs=1) as in_pool,
            tc.tile_pool(name="out", bufs=5) as out_pool,
            tc.tile_pool(name="tmp", bufs=4) as tmp_pool,
        ):
            # Persistent input slots with 2 leading zero columns so the
            # shifted views u[t-1], u[t-2] fall out of plain column offsets.
            # The zero columns are written ONCE here; the per-iteration DMAs
            # only write cols [2:], so no DMA ever waits on a memset.
            in_slots = [
                in_pool.tile(
                    [P, K + 2], mybir.dt.float32, tag=f"xin{j}", name=f"xin{j}"
                )
                for j in range(N_SLOTS)
            ]
            for j in range(N_SLOTS):
                nc.vector.memset(in_slots[j][:, 0:2], 0.0)

            for g in range(n_groups):
                xin = in_slots[g % N_SLOTS]
                rows = slice(g * P, (g + 1) * P)
                # Input DMAs on the SP HWDGE ring (Sync sequencer).
                nc.sync.dma_start(xin[:, 2 : 2 + K], x[rows, :])

                a = xin[:, 2 : 2 + K]  # u[t]
                b = xin[:, 1 : 1 + K]  # u[t-1]
                c = xin[:, 0:K]        # u[t-2]

                out = out_pool.tile(
                    [P, N_OUT * K], mybir.dt.float32, tag="out", name="out"
                )
                even = out[:, 0 : N_OUT * K : 2]
                odd = out[:, 1 : N_OUT * K : 2]

                # p = a - c in {-1,0,1}; out0 = p^2 = a ^ c
                p = tmp_pool.tile([P, K], mybir.dt.float32, tag="p", name="p")
                nc.vector.tensor_tensor(p[:], a, c, mybir.AluOpType.subtract)
                nc.scalar.square(even, p[:])

                # q = out0 - b in {-1,0,1}; out1 = q^2 = out0 ^ b
                # (reuses p's buffer: p is dead once the first square ran)
                nc.vector.tensor_tensor(p[:], even, b, mybir.AluOpType.subtract)
                nc.scalar.square(odd, p[:])

                # Output DMAs on the SWDGE path (GpSimd sequencer) so a
                # stalled input-DMA trigger never blocks a ready output DMA
                # (and vice versa) — the two streams issue independently.
                nc.gpsimd.dma_start(y[rows, :], out[:])

    nc.compile()
    return nc


def _get_nc():
    if "nc" not in _compiled:
        _compiled["nc"] = _build_nc()
    return _compiled["nc"]


def kernel(**inputs) -> np.ndarray:
    from concourse.bass_utils import run_bass_kernel_spmd

    x_full = np.ascontiguousarray(np.asarray(inputs["inputs"], dtype=np.float32))
    assert x_full.shape == (B, K), x_full.shape

    nc = _get_nc()
    in_maps = [
        {"x": x_full[i * SHARD_B : (i + 1) * SHARD_B]} for i in range(N_CORES)
    ]
    res = run_bass_kernel_spmd(nc, in_maps, core_ids=list(range(N_CORES)))
    out = np.concatenate([r["y"] for r in res.results], axis=0)
    return np.ascontiguousarray(out, dtype=np.float32)



# revision 3
# speedup vs baseline: 2.9181x; 2.9181x over previous
"""Trainium2 Bass kernel for a rate-1/2, constraint-length-3 feedforward
convolutional encoder (generator polynomials "101" and "111", MSB-first).

The trellis scan in the reference collapses to elementwise XORs of shifted
input bits (zero initial state):

    out0[t] = u[t] ^ u[t-2]            (poly "101")
    out1[t] = u[t] ^ u[t-1] ^ u[t-2]   (poly "111")

with the codeword interleaved time-major: y[:, 2t] = out0[t], y[:, 2t+1] = out1[t].

The kernel is pure HBM traffic, so the device works on uint8 tensors (the
bits are 0/1 — exact in u8; the host casts at the numpy boundary): 6.25 MiB
of device traffic per core instead of 24 MiB in f32.

Layout: block-transposed, slot-major. SBUF partition p holds an 18-slot
window u[16p-2 .. 16p+16) of all 1024 codewords of the core (slot k is a
contiguous 1024-byte run holding bit u[16p-2+k] of every codeword). The
u[t-1] / u[t-2] shifts become slot offsets (multiples of 1024 bytes), so
every XOR runs full-width on uint32 lanes — 4 codewords per lane-cycle —
with no partition-offset or byte-misaligned access. The 2-slot overlap
between consecutive partitions (+12.5% input re-read) replaces any
cross-partition dependency; the encoder's zero initial state is two
host-padded zero slots in partition 0.

Sharding: pure data parallel over the batch dim across 8 NeuronCores.
"""

import numpy as np

N_CORES = 8
B, K = 8192, 2048
N_OUT = 2
SHARD_B = B // N_CORES          # 1024 codewords per core
W = SHARD_B // 4                # 256 uint32 words per slot (4 codewords each)
P = 128                         # SBUF partitions
SLOTS = K // P                  # 16 output slots per partition
IN_SLOTS = SLOTS + 2            # +2 overlap slots for the u[t-1]/u[t-2] taps
GROUPS = 4                      # slot groups for pipelining
GS = SLOTS // GROUPS            # 4 output slots per group

_compiled = {}


def _build_nc():
    import concourse.bass as bass  # noqa: F401
    import concourse.tile as tile
    from concourse import bacc, mybir

    nc = bacc.Bacc(
        "TRN2",
        target_bir_lowering=False,
        debug=False,
        enable_asserts=False,
    )
    # x row p = 18 slots x 1024 codeword-bits: u[16p-2+k][b] at word k*W + b/4
    x = nc.dram_tensor(
        "x", [P, IN_SLOTS * W], mybir.dt.uint32, kind="ExternalInput"
    ).ap()
    # y row p = [j, k, b]: bit j of symbol t=16p+k -> word j*SLOTS*W + k*W + b/4
    y = nc.dram_tensor(
        "y", [P, N_OUT * SLOTS * W], mybir.dt.uint32, kind="ExternalOutput"
    ).ap()

    xor = mybir.AluOpType.bitwise_xor

    with tile.TileContext(nc) as tc:
        with (
            tc.tile_pool(name="xin", bufs=1) as in_pool,
            tc.tile_pool(name="out", bufs=1) as out_pool,
        ):
            xin = in_pool.tile([P, IN_SLOTS * W], mybir.dt.uint32, tag="xin", name="xin")
            out = out_pool.tile(
                [P, N_OUT * SLOTS * W], mybir.dt.uint32, tag="out", name="out"
            )

            # Input sub-DMAs on the SP HWDGE ring: slot ranges [0,6), [6,10),
            # [10,14), [14,18). Compute group g needs input slots [4g, 4g+6),
            # so group 0 only waits on the first sub-DMA.
            bounds = [0, GS + 2] + [GS * g + 2 for g in range(2, GROUPS + 1)]
            for i in range(GROUPS):
                s0, s1 = bounds[i], bounds[i + 1]
                nc.sync.dma_start(
                    xin[:, s0 * W : s1 * W], x[:, s0 * W : s1 * W]
                )

            for g in range(GROUPS):
                k0 = GS * g
                a = xin[:, (k0 + 2) * W : (k0 + 2 + GS) * W]  # u[t]
                b = xin[:, (k0 + 1) * W : (k0 + 1 + GS) * W]  # u[t-1]
                c = xin[:, k0 * W : (k0 + GS) * W]            # u[t-2]
                out0 = out[:, k0 * W : (k0 + GS) * W]
                out1 = out[:, (SLOTS + k0) * W : (SLOTS + k0 + GS) * W]

                nc.vector.tensor_tensor(out0, a, c, xor)
                nc.vector.tensor_tensor(out1, out0, b, xor)

                # Output DMAs on the SWDGE path (GpSimd sequencer) so input
                # and output streams trigger independently.
                nc.gpsimd.dma_start(
                    y[:, k0 * W : (k0 + GS) * W], out0
                )
                nc.gpsimd.dma_start(
                    y[:, (SLOTS + k0) * W : (SLOTS + k0 + GS) * W], out1
                )

    nc.compile()
    return nc


def _get_nc():
    if "nc" not in _compiled:
        _compiled["nc"] = _build_nc()
    return _compiled["nc"]


def _shard_inputs(x_full: np.ndarray) -> list[dict]:
    """Cast the 0/1 float input to uint8 and build the per-core block-
    transposed, slot-overlapped layout (see module docstring)."""
    xu8 = x_full.astype(np.uint8)            # exact: values are 0.0 / 1.0
    in_maps = []
    for i in range(N_CORES):
        xt = np.ascontiguousarray(xu8[i * SHARD_B : (i + 1) * SHARD_B].T)
        blk = xt.reshape(P, SLOTS, SHARD_B)  # [p, k, b] = u[16p+k][b]
        xb = np.zeros((P, IN_SLOTS, SHARD_B), np.uint8)
        xb[:, 2:] = blk
        xb[1:, :2] = blk[:-1, SLOTS - 2 :]   # u[16p-2], u[16p-1]
        in_maps.append({"x": xb.reshape(P, IN_SLOTS * SHARD_B).view(np.uint32)})
    return in_maps


def _gather_output(results) -> np.ndarray:
    """Un-transpose and interleave: y[p, j, k, b] -> out[b, 2*(16p+k)+j]."""
    out = np.empty((B, N_OUT * K), np.float32)
    for i, r in enumerate(results):
        y_t = r["y"].view(np.uint8).reshape(P, N_OUT, SLOTS, SHARD_B)
        out[i * SHARD_B : (i + 1) * SHARD_B] = (
            y_t.transpose(3, 0, 2, 1).reshape(SHARD_B, N_OUT * K)
        )
    return out


def kernel(**inputs) -> np.ndarray:
    from concourse.bass_utils import run_bass_kernel_spmd

    x_full = np.ascontiguousarray(np.asarray(inputs["inputs"], dtype=np.float32))
    assert x_full.shape == (B, K), x_full.shape

    nc = _get_nc()
    in_maps = _shard_inputs(x_full)
    res = run_bass_kernel_spmd(nc, in_maps, core_ids=list(range(N_CORES)))
    return _gather_output(res.results)
